# revision 19
# baseline (speedup 1.0000x reference)
"""Causal self-attention Trainium2 Bass kernel.

Problem: nn_CausalSelfAttention (B=2, L=2048, D=1024, H=16 heads, Khd=64).

Sharding (8 cores): data-parallel over B (2 way) x tensor-parallel over
heads (4 way, 4 heads/core).  Each core computes
  qkv_local = x_b @ W_attn_local.T          (c_attn column-sharded)
  attn_local = causal_attention(q,k,v)      (4 heads)
  y_partial  = attn_local @ W_proj_local.T  (c_proj row-sharded)
and the host sums the 4 partials per batch (the row-parallel unshard).

Device pipeline (v2):
  - All DRAM inputs are laid out partition-major ([128, chunk, cols]) so a
    column strip of every contraction chunk is ONE dma with 512B+ runs,
    ordered so the first matmul group is fed after ~2MB.
  - QKV matmuls in float32r; q/k/v stored bf16.  Scores, PV and the output
    projection run fully in bf16 (fp32 PSUM), which avoids the fp32r
    small-N penalty on the causal diagonal.
  - Scores are computed transposed scT[key, query]; softmax denominator via
    an appended ones-row in V; exp on ScalarE (scale fused), causal mask on
    the diagonal blocks via a single strided-tri multiply per (head, round).
  - PSUM: 2x[128,1024] score tiles, 2 PV accumulators, 2 flex banks for the
    qkv/projection granules.
  - Scheduling: one background granule (qkv column block, v chunk, or
    projection tile) is emitted per attention round so the PE absorbs the
    ScalarE exp latency and the projection streams behind attention.
"""

import math

import numpy as np

B, L, D, H = 2, 2048, 1024, 16
KHD = D // H  # 64 head dim
NCORES = 8
HPC = 4  # heads per core
FQK = 2 * HPC * KHD  # 512 q+k local features
FV = HPC * KHD  # 256 v local features
FQKV = FQK + FV  # 768
DK = D // 128  # 8 contraction chunks
LC = L // 128  # 16 row chunks
NJ = L // 512  # 4 qrow blocks
SCALE = 1.0 / math.sqrt(KHD)

_CACHE = {}


def _build(has_bqk: bool, has_bv: bool, has_bp: bool, reps: int = 1):
    import concourse.bass as bass
    import concourse.mybir as mybir
    import concourse.tile as tile
    from concourse import bacc

    f32 = mybir.dt.float32
    f32r = mybir.dt.float32r
    bf16 = mybir.dt.bfloat16

    nc = bacc.Bacc(None, target_bir_lowering=False)
    # partition-major layouts: [p, chunk, cols]
    xT_d = nc.declare_dram_parameter("xT", [128, DK, L], f32r, isOutput=False)
    wq_d = nc.declare_dram_parameter("wqkvT", [128, DK, FQKV], f32r, isOutput=False)
    wp_d = nc.declare_dram_parameter("wpT", [128, 2, D], bf16, isOutput=False)
    tri_d = nc.declare_dram_parameter("tri", [128, 256], bf16, isOutput=False)
    if has_bqk:
        bqk_d = nc.declare_dram_parameter("bqk", [128, FQK // 128], f32, isOutput=False)
    if has_bv:
        bv_d = nc.declare_dram_parameter("bv", [1, FV], f32r, isOutput=False)
        onesr_d = nc.declare_dram_parameter("onesr", [1, 128], f32r, isOutput=False)
    if has_bp:
        bp_d = nc.declare_dram_parameter("bp", [1, D], bf16, isOutput=False)
        onesb_d = nc.declare_dram_parameter("onesb", [1, 128], bf16, isOutput=False)
    # y partition-major: y[p, lc, d] = out[lc*128+p, d]
    y_d = nc.declare_dram_parameter("y", [128, LC, D], bf16, isOutput=True)

    with nc.allow_low_precision(reason="tf32/bf16 matmul pipeline"), tile.TileContext(nc) as tc:
        with (
            tc.tile_pool(name="persist", bufs=1) as persist,
            tc.tile_pool(name="work", bufs=3) as work,
            tc.tile_pool(name="ps_sc", bufs=2, space="PSUM") as ps_sc,
            tc.tile_pool(name="flex", bufs=2, space="PSUM") as flex,
        ):
            for _rep in range(reps):
                # ---- persistent SBUF tensors ----
                xT = persist.tile([128, DK, L], f32r, name="xT_sb", tag="xT_sb")
                wq = persist.tile([128, DK, FQKV], f32r, name="wq_sb", tag="wq_sb")
                wp = persist.tile([128, 2, D], bf16, name="wp_sb", tag="wp_sb")
                qT = persist.tile([128, 2, L], bf16, name="qT", tag="qT")
                kT = persist.tile([128, 2, L], bf16, name="kT", tag="kT")
                vA = persist.tile([128, LC, HPC, KHD + 1], bf16, name="vA", tag="vA")
                aT = persist.tile([128, NJ, 2, 512], bf16, name="aT", tag="aT")
                tri = persist.tile([128, 2, 128], bf16, name="tri_sb", tag="tri_sb")

                # ---- DMA schedule (ordered by first use; sync=HWDGE) ----
                nc.sync.dma_start(out=wq[:, :, 0:128], in_=wq_d[:, :, 0:128])  # q pl0
                nc.sync.dma_start(out=xT[:, :, 0:256], in_=xT_d[:, :, 0:256])
                nc.sync.dma_start(out=wq[:, :, 256:384], in_=wq_d[:, :, 256:384])  # k pl0
                nc.sync.dma_start(out=xT[:, :, 256:512], in_=xT_d[:, :, 256:512])
                nc.sync.dma_start(out=wq[:, :, 512:768], in_=wq_d[:, :, 512:768])  # v
                nc.sync.dma_start(out=wq[:, :, 128:256], in_=wq_d[:, :, 128:256])  # q pl1
                nc.sync.dma_start(out=wq[:, :, 384:512], in_=wq_d[:, :, 384:512])  # k pl1
                nc.sync.dma_start(out=xT[:, :, 512:768], in_=xT_d[:, :, 512:768])
                nc.sync.dma_start(out=xT[:, :, 768:1024], in_=xT_d[:, :, 768:1024])
                nc.sync.dma_start(out=tri, in_=tri_d[:])
                nc.sync.dma_start(out=wp, in_=wp_d[:])
                nc.sync.dma_start(out=xT[:, :, 1024:1536], in_=xT_d[:, :, 1024:1536])
                nc.sync.dma_start(out=xT[:, :, 1536:2048], in_=xT_d[:, :, 1536:2048])
                nc.gpsimd.memset(vA[:, :, :, KHD:KHD + 1], 1.0)
                if has_bqk:
                    bqk_sb = persist.tile([128, FQK // 128], f32)
                    nc.sync.dma_start(out=bqk_sb, in_=bqk_d[:])
                if has_bv:
                    bv_sb = persist.tile([1, FV], f32r)
                    nc.sync.dma_start(out=bv_sb, in_=bv_d[:])
                    onesr_sb = persist.tile([1, 128], f32r)
                    nc.sync.dma_start(out=onesr_sb, in_=onesr_d[:])
                if has_bp:
                    bp_sb = persist.tile([1, D], bf16)
                    nc.sync.dma_start(out=bp_sb, in_=bp_d[:])
                    onesb_sb = persist.tile([1, 128], bf16)
                    nc.sync.dma_start(out=onesb_sb, in_=onesb_d[:])

                ybig = {}
                ydone = {}

                # ---- emission helpers ----
                def qk_copy(m, ps, n0, n1):
                    # PSUM qkv chunk -> bf16 qT/kT slice (with optional bias)
                    dst = qT if m < 2 else kT
                    pl = m % 2
                    if has_bqk:
                        nc.scalar.activation(
                            dst[:, pl, n0:n1], ps,
                            mybir.ActivationFunctionType.Copy,
                            bias=bqk_sb[:, m:m + 1],
                        )
                    else:
                        nc.vector.tensor_copy(out=dst[:, pl, n0:n1], in_=ps)

                def qk_mm(ps, psl, m, n0, n1, k):
                    nc.tensor.matmul(
                        ps[:, psl],
                        wq[:, k, m * 128:(m + 1) * 128],
                        xT[:, k, n0:n1],
                        start=(k == 0),
                        stop=(k == DK - 1),
                    )

                def emit_qk_sc_granules(psa, psb, m_a, m_b, ranges):
                    # qk chunks m_a, m_b granules into the 2 sc-pool tiles,
                    # interleaved to follow the x strip arrivals.
                    for g0, g1 in ranges:
                        for ps, m in ((psa, m_a), (psb, m_b)):
                            for k in range(DK):
                                qk_mm(ps, slice(g0, g1), m, g0, g1, k)

                def emit_qk_flex(m, n0):
                    # one [128, 512] flex granule for qk chunk m, cols n0:n0+512
                    ps = flex.tile([128, 512], f32, tag="fx", name="psqkf")
                    for k in range(DK):
                        qk_mm(ps, slice(0, 512), m, n0, n0 + 512, k)
                    qk_copy(m, ps, n0, n0 + 512)

                def emit_v2(lc0):
                    # v chunks lc0, lc0+1 through one [128,512] flex granule
                    ps = flex.tile([128, 512], f32, tag="fx", name="psv")
                    for i in range(2):
                        sl = slice(i * 256, i * 256 + FV)
                        for k in range(DK):
                            nc.tensor.matmul(
                                ps[:, sl],
                                xT[:, k, (lc0 + i) * 128:(lc0 + i + 1) * 128],
                                wq[:, k, FQK:FQKV],
                                start=(k == 0),
                                stop=(k == DK - 1) and not has_bv,
                            )
                        if has_bv:
                            nc.tensor.matmul(
                                ps[:, sl], onesr_sb[0:1, :], bv_sb,
                                start=False, stop=True,
                            )
                    nc.vector.tensor_copy(
                        out=vA[:, lc0:lc0 + 2, :, 0:KHD],
                        in_=ps.rearrange("p (l h k) -> p l h k", l=2, h=HPC),
                    )

                def emit_proj(j, lq, half, cpeng="v"):
                    # one [128,512] projection tile -> ybig slice (-> dram when
                    # all 4 row chunks of this (j, half) are done)
                    if (j, half) not in ybig:
                        ybig[(j, half)] = work.tile(
                            [128, 4, 512], bf16, tag="ybig", name="ybig", bufs=3
                        )
                        ydone[(j, half)] = set()
                    sl = slice(half * 512, (half + 1) * 512)
                    psy = flex.tile([128, 512], f32, tag="fx", name="psy")
                    for kc in range(2):
                        nc.tensor.matmul(
                            psy,
                            aT[:, j, kc, lq * 128:(lq + 1) * 128],
                            wp[:, kc, sl],
                            start=(kc == 0),
                            stop=(kc == 1) and not has_bp,
                        )
                    if has_bp:
                        nc.tensor.matmul(
                            psy, onesb_sb[0:1, :], bp_sb[0:1, sl],
                            start=False, stop=True,
                        )
                    yb = ybig[(j, half)]
                    if cpeng == "s":
                        nc.scalar.activation(
                            yb[:, lq, :], psy, mybir.ActivationFunctionType.Copy
                        )
                    else:
                        nc.vector.tensor_copy(out=yb[:, lq, :], in_=psy)
                    done = ydone[(j, half)]
                    done.add(lq)
                    if len(done) == 4:
                        nc.sync.dma_start(
                            out=y_d[:, 4 * j:4 * j + 4, sl], in_=yb
                        )

                def emit_att_pair(j, h0, bg, n_bg=1, bg_cap=99):
                    # two heads (same q/k plane) in lockstep; PV one round
                    # behind the scores; n_bg background granules per round.
                    pl = h0 // 2
                    pos = [(h0 % 2) * 64, ((h0 + 1) % 2) * 64]
                    heads = [h0, h0 + 1]
                    outTs = [
                        flex.tile([KHD + 1, 512], f32, tag="outT", name="outT")
                        for _ in range(2)
                    ]
                    qrs = slice(j * 512, (j + 1) * 512)
                    last_c = 4 * j + 3
                    rounds = [("below", cp) for cp in range(0, 4 * j, 2)]
                    rounds += [("diag", 0), ("diag", 2)]
                    pending = []

                    def flush_pending():
                        for hh, parts, ex in pending:
                            for c, exsl, n0 in parts:
                                nc.tensor.matmul(
                                    outTs[hh][:, n0:512],
                                    vA[:, c, heads[hh], :],
                                    ex[:, exsl],
                                    start=(c == 0),
                                    stop=(c == last_c),
                                )
                        pending.clear()

                    for kind, arg in rounds:
                        new_pending = []
                        for hh in range(2):
                            po = pos[hh]
                            sc = ps_sc.tile([128, 1024], f32, tag="sc", name="sc")
                            if kind == "below":
                                cp = arg
                                for half in range(2):
                                    c = cp + half
                                    nc.tensor.matmul(
                                        sc[:, half * 512:(half + 1) * 512],
                                        kT[po:po + 64, pl, c * 128:(c + 1) * 128],
                                        qT[po:po + 64, pl, qrs],
                                        start=True,
                                        stop=True,
                                    )
                                ex = work.tile([128, 1024], bf16, tag="expT", name="ex", bufs=6)
                                nc.scalar.activation(
                                    ex, sc,
                                    mybir.ActivationFunctionType.Exp, scale=SCALE,
                                )
                                parts = [
                                    (cp, slice(0, 512), 0),
                                    (cp + 1, slice(512, 1024), 0),
                                ]
                            else:
                                i0 = arg
                                ws = [512 - 128 * (i0 + di) for di in range(2)]
                                wtot = ws[0] + ws[1]
                                for di in range(2):
                                    c = 4 * j + i0 + di
                                    n0 = 128 * (i0 + di)
                                    nc.tensor.matmul(
                                        sc[:, ws[0] * di:ws[0] * di + ws[di]],
                                        kT[po:po + 64, pl, c * 128:(c + 1) * 128],
                                        qT[po:po + 64, pl, j * 512 + n0:(j + 1) * 512],
                                        start=True,
                                        stop=True,
                                    )
                                ex = work.tile([128, 1024], bf16, tag="expT", name="ex", bufs=6)
                                nc.scalar.activation(
                                    ex[:, 0:wtot], sc[:, 0:wtot],
                                    mybir.ActivationFunctionType.Exp, scale=SCALE,
                                )
                                # causal mask: one strided multiply hits both
                                # diagonal 128-blocks (at offsets 0 and ws[0])
                                exm = ex[:, 0:2 * ws[0]].rearrange(
                                    "p (b w) -> p b w", b=2
                                )[:, :, 0:128]
                                nc.vector.tensor_mul(exm, exm, tri)
                                parts = [
                                    (4 * j + i0, slice(0, ws[0]), 128 * i0),
                                    (
                                        4 * j + i0 + 1,
                                        slice(ws[0], wtot),
                                        128 * (i0 + 1),
                                    ),
                                ]
                            new_pending.append((hh, parts, ex))
                        # background granules between scores and pending PV
                        for _ in range(n_bg):
                            if bg_cap > 0:
                                nxt = next(bg, None)
                                if nxt is not None:
                                    bg_cap -= 1
                                    nxt()
                        flush_pending()
                        pending.extend(new_pending)
                    flush_pending()
                    # normalize: aT[f, qrow] = outT[f, qrow] / outT[64, qrow]
                    last_pair = (j == NJ - 1) and (h0 == 2)
                    bcs = []
                    for hh in range(2):
                        recip = work.tile([1, 512], f32r, tag="recip", name="recip", bufs=2)
                        nc.vector.reciprocal(recip, outTs[hh][KHD:KHD + 1, :])
                        bc_sb = work.tile([64, 512], f32r, tag="bcsb", name="bc_sb", bufs=2)
                        nc.gpsimd.partition_broadcast(bc_sb, recip)
                        bcs.append(bc_sb)
                    if last_pair:
                        # per-128-query-slice muls, hh interleaved: the tail
                        # projection tiles unblock as each lq slice lands
                        for q0 in range(0, 512, 128):
                            for hh in range(2):
                                nc.vector.tensor_mul(
                                    aT[pos[hh]:pos[hh] + 64, j, pl, q0:q0 + 128],
                                    outTs[hh][0:KHD, q0:q0 + 128],
                                    bcs[hh][:, q0:q0 + 128],
                                )
                    else:
                        for hh in range(2):
                            nc.vector.tensor_mul(
                                aT[pos[hh]:pos[hh] + 64, j, pl, :],
                                outTs[hh][0:KHD, :], bcs[hh],
                            )

                # ---- S0 (minimal): q/k plane0 cols 0:512 + v chunks
                # 0..3; everything else streams as background granules ----
                psa = ps_sc.tile([128, 1024], f32, tag="sc", name="psqka")
                psb = ps_sc.tile([128, 1024], f32, tag="sc", name="psqkb")
                emit_qk_sc_granules(psa, psb, 0, 2, [(0, 256), (256, 512)])
                qk_copy(0, psa[:, 0:512], 0, 512)
                qk_copy(2, psb[:, 0:512], 0, 512)
                emit_v2(0)
                emit_v2(2)

                # ---- background queue: one granule per attention round ----
                def G(f, *a):
                    return lambda: f(*a)

                bg_items = [
                    # (0,0) 2 rounds x2
                    G(emit_qk_flex, 1, 0), G(emit_qk_flex, 3, 0),
                    G(emit_qk_flex, 0, 512), G(emit_qk_flex, 2, 512),
                    # (0,2) 2 rounds x2
                    G(emit_qk_flex, 1, 512), G(emit_qk_flex, 3, 512),
                    G(emit_v2, 4), G(emit_v2, 6),
                    # (1,0) 4 rounds
                    G(emit_qk_flex, 0, 1024), G(emit_qk_flex, 2, 1024),
                    G(emit_qk_flex, 1, 1024), G(emit_qk_flex, 3, 1024),
                    # (1,2) 4 rounds
                    G(emit_qk_flex, 0, 1536), G(emit_qk_flex, 2, 1536),
                    G(emit_qk_flex, 1, 1536), G(emit_qk_flex, 3, 1536),
                    # (2,0) 6 rounds
                    G(emit_v2, 8), G(emit_v2, 10),
                    G(emit_proj, 0, 0, 0), G(emit_proj, 0, 1, 0),
                    G(emit_proj, 0, 2, 0), G(emit_proj, 0, 3, 0),
                    # (2,2) 6 rounds
                    G(emit_v2, 12), G(emit_v2, 14),
                    G(emit_proj, 0, 0, 1), G(emit_proj, 0, 1, 1),
                    G(emit_proj, 0, 2, 1), G(emit_proj, 0, 3, 1),
                    # (3,0) 8 rounds
                    G(emit_proj, 1, 0, 0), G(emit_proj, 1, 1, 0),
                    G(emit_proj, 1, 2, 0), G(emit_proj, 1, 3, 0),
                    G(emit_proj, 1, 0, 1), G(emit_proj, 1, 1, 1),
                    G(emit_proj, 1, 2, 1), G(emit_proj, 1, 3, 1),
                    # (3,2) 8 rounds
                    G(emit_proj, 2, 0, 0), G(emit_proj, 2, 1, 0),
                    G(emit_proj, 2, 2, 0), G(emit_proj, 2, 3, 0),
                    G(emit_proj, 2, 0, 1), G(emit_proj, 2, 1, 1),
                    G(emit_proj, 2, 2, 1), G(emit_proj, 2, 3, 1),
                ]
                bg = iter(bg_items)

                for j in range(NJ):
                    nb = 2 if j == 0 else 1
                    cap = 99
                    emit_att_pair(j, 0, bg, n_bg=nb)
                    emit_att_pair(j, 2, bg, n_bg=nb, bg_cap=cap)

                # drain leftovers, then the tail projection of block 3:
                # phase 1 opens kc=0 groups (plane-0 attn, already normalized)
                # across 6 psum slots while the last normalize chain runs;
                # phase 2 closes with kc=1 + copies alternating DVE/ScalarE.
                for item in bg:
                    item()
                ybig[(3, 0)] = work.tile([128, 4, 512], bf16, tag="ybig", name="ybig", bufs=3)
                ybig[(3, 1)] = work.tile([128, 4, 512], bf16, tag="ybig", name="ybig", bufs=3)
                sct = [ps_sc.tile([128, 1024], f32, tag="sc", name="sctail") for _ in range(2)]
                plan = [(lq, half) for half in range(2) for lq in range(4)]
                handles = []
                for idx, (lq, half) in enumerate(plan[:6]):
                    if idx < 4:
                        ps = sct[idx // 2][:, (idx % 2) * 512:(idx % 2 + 1) * 512]
                    else:
                        ps = flex.tile([128, 512], f32, tag="fx", name="psyt")
                    nc.tensor.matmul(
                        ps, aT[:, 3, 0, lq * 128:(lq + 1) * 128],
                        wp[:, 0, half * 512:(half + 1) * 512],
                        start=True, stop=False,
                    )
                    handles.append((ps, lq, half))

                def tail_close(ps, lq, half, cpeng):
                    sl = slice(half * 512, (half + 1) * 512)
                    nc.tensor.matmul(
                        ps, aT[:, 3, 1, lq * 128:(lq + 1) * 128],
                        wp[:, 1, sl],
                        start=False, stop=not has_bp,
                    )
                    if has_bp:
                        nc.tensor.matmul(
                            ps, onesb_sb[0:1, :], bp_sb[0:1, sl],
                            start=False, stop=True,
                        )
                    yb = ybig[(3, half)]
                    if cpeng == "s":
                        nc.scalar.activation(
                            yb[:, lq, :], ps, mybir.ActivationFunctionType.Copy
                        )
                    else:
                        nc.vector.tensor_copy(out=yb[:, lq, :], in_=ps)
                    done = ydone.setdefault((3, half), set())
                    done.add(lq)
                    if done >= {0, 1} and (half, 0) not in ytail_sent:
                        ytail_sent.add((half, 0))
                        nc.sync.dma_start(
                            out=y_d[:, 12:14, sl], in_=yb[:, 0:2, :]
                        )
                    if done >= {2, 3} and (half, 1) not in ytail_sent:
                        ytail_sent.add((half, 1))
                        nc.sync.dma_start(
                            out=y_d[:, 14:16, sl], in_=yb[:, 2:4, :]
                        )

                ytail_sent = set()
                for i, (ps, lq, half) in enumerate(handles):
                    tail_close(ps, lq, half, "sv"[i % 2])
                for i, (lq, half) in enumerate(plan[6:]):
                    ps = flex.tile([128, 512], f32, tag="fx", name="psyt2")
                    nc.tensor.matmul(
                        ps, aT[:, 3, 0, lq * 128:(lq + 1) * 128],
                        wp[:, 0, half * 512:(half + 1) * 512],
                        start=True, stop=False,
                    )
                    tail_close(ps, lq, half, "sv"[i % 2])

    nc.compile()
    return nc


def make_in_maps(inputs):
    """Build the 8 per-core input maps from full inputs."""
    import ml_dtypes

    input_BLD = np.asarray(inputs["input_BLD"], dtype=np.float32)
    W_attn = np.asarray(inputs["W_attn"], dtype=np.float32)
    b_attn = np.asarray(inputs["b_attn"], dtype=np.float32)
    W_proj = np.asarray(inputs["W_proj"], dtype=np.float32)
    b_proj = np.asarray(inputs["b_proj"], dtype=np.float32)

    has_bqk = bool(np.any(b_attn[: 2 * D]))
    has_bv = bool(np.any(b_attn[2 * D:]))
    has_bp = bool(np.any(b_proj))

    bf16 = ml_dtypes.bfloat16
    tri1 = (np.arange(128)[None, :] >= np.arange(128)[:, None]).astype(bf16)
    tri = np.ascontiguousarray(np.concatenate([tri1, tri1], axis=1))
    in_maps = []
    for c in range(NCORES):
        b, t = divmod(c, 4)
        hs = t * HPC * KHD  # feature offset of this core's heads
        w_loc = np.concatenate(
            [
                W_attn[hs:hs + FV],  # q rows
                W_attn[D + hs:D + hs + FV],  # k rows
                W_attn[2 * D + hs:2 * D + hs + FV],  # v rows
            ],
            axis=0,
        )  # [768, 1024]
        # partition-major: [128, chunk, cols]
        xT = input_BLD[b].T.reshape(DK, 128, L).transpose(1, 0, 2)
        wqT = w_loc.T.reshape(DK, 128, FQKV).transpose(1, 0, 2)
        wpT = W_proj[:, hs:hs + FV].T.reshape(2, 128, D).transpose(1, 0, 2)
        m = {
            "xT": np.ascontiguousarray(xT),
            "wqkvT": np.ascontiguousarray(wqT),
            "wpT": np.ascontiguousarray(wpT.astype(bf16)),
            "tri": tri,
        }
        if has_bqk:
            bqk = np.concatenate([b_attn[hs:hs + FV], b_attn[D + hs:D + hs + FV]])
            m["bqk"] = np.ascontiguousarray(bqk.reshape(FQK // 128, 128).T)
        if has_bv:
            m["bv"] = b_attn[2 * D + hs:2 * D + hs + FV][None, :].copy()
            m["onesr"] = np.ones((1, 128), np.float32)
        if has_bp:
            m["bp"] = (b_proj / 4.0)[None, :].astype(bf16)
            m["onesb"] = np.ones((1, 128), bf16)
        in_maps.append(m)
    return in_maps, (has_bqk, has_bv, has_bp)


def kernel(input_BLD, W_attn, b_attn, W_proj, b_proj):
    in_maps, key = make_in_maps(dict(
        input_BLD=input_BLD, W_attn=W_attn, b_attn=b_attn,
        W_proj=W_proj, b_proj=b_proj,
    ))
    if key not in _CACHE:
        _CACHE[key] = _build(*key)
    nc = _CACHE[key]

    from concourse.bass_utils import run_bass_kernel_spmd

    globals()["_last_in_maps"] = in_maps
    res = run_bass_kernel_spmd(nc, in_maps, list(range(NCORES)))
    globals()["_LAST_RESULTS"] = res
    out = np.empty((B, L, D), dtype=np.float32)
    for b in range(B):
        acc = res.results[4 * b]["y"].astype(np.float32)
        for t in range(1, 4):
            acc = acc + res.results[4 * b + t]["y"]
        # y is partition-major [128, LC, D] -> [L, D]
        out[b] = acc.transpose(1, 0, 2).reshape(L, D)
    return out
